# revision 40
# baseline (speedup 1.0000x reference)
"""Trainium2 Bass kernel for ChannelProjector2D: out[b,h,w,o] = x[b,h,w,c] @ W[c,o].

Strategy (data-parallel, one batch image per NeuronCore; int8 I/O):
  - Error gate is rel_err < 2e-2. bf16 I/O gives 2.9e-3 but is DMA-bound at
    ~150-173us (51.4 MB/core through the SDMA engines at ~330 GB/s). The
    binding resource is SDMA-processed bytes, so both directions go int8:
      x: host-quantized  xq = clip(rint(x*31.75), -127, 127)  (s_x = 127/4)
      W: host-folded     Wq[c,o] = bf16(W[c,o] * t_o / s_x),  t_o = 127/(5*sigma_o)
      out: PSUM (= t_o * out_o) stored rint->int8 saturating; host divides by t_o.
    Probed on HW: ACT/DVE f32->int8 stores round-to-nearest-even + saturate,
    and SWDGE cast-DMA int8->bf16 is an exact value cast. rel_err 1.486e-2
    (measured HW == numpy sim on the exact generator data; 46 sat elts).
  - Per core: int8 x rows channels-major (c on partitions, 2 chunks of 128).
    Alternating 1024-row groups: (a) HWDGE int8 DMA + one whole-group
    int8->bf16 cast on ACT or DVE (single writer per tile -- chunked/multi-
    writer casts inflate PE sem-wait time ~45ns/MM, measured); (b) gpsimd
    SWDGE cast-DMA straight to bf16 (costs 2B/elt of SDMA-processed bytes
    but zero engine time). bf16 matmuls (2cc x 2oc per 512 rows, pairs of
    subgroups interleaved across 2-bank PSUM tiles) accumulate f32; a single
    [128,2,512] PSUM->SBUF int8 copy per subgroup alternates ACT/DVE
    (act_copy slots balance the 1.2 vs 0.96 GHz engines). Out-DMA int8 on
    the SWDGE ring; the last 3 blocks drain on the HWDGE rings (lower
    completion latency, lets gpsimd's 5us dge_drain overlap). ~10 dummy
    matmuls on zeroed SBUF during boot release the PE HAM clock gate
    (1.2->2.4 GHz) before real work arrives; small front blocks prime the
    pipeline.
  - Measured (NTFF, 8 cores): ~109-110us mean, ~111-112.5us max-core
    (swdge_every=4 trades ~0.3us mean for a tighter max under co-tenant HBM
    contention), vs 146-157us for the bf16 baseline. Breakdown: PE busy
    ~90us (384 MM x ~225ns, N=512 warm + LDW residual; 4 cyc/row = 83.6us
    floor), ACT ~80, DVE ~75; boot ~12us (Tile preamble ~7 + first DMA) and
    tail ~9.5us (copy + DMA receipt ~2 + 5-engine exit-barrier ring ~4.5)
    bracket the span. fp8 x fails the gate (2.7e-2); int8 is the traffic
    floor since matmul needs >=8-bit x, and PE at 4 cyc/row is then the
    wall. Merging the per-subgroup PSUM copies into one 4-bank copy halves
    PSUM recycling granularity and regresses ~20us — keep 2-bank tiles.
"""

import numpy as np
import ml_dtypes

BF16 = ml_dtypes.bfloat16

P = 128
CIN = 256
COUT = 256
B, H, Wdim = 8, 224, 224
M_CORE = H * Wdim          # 50176 rows per core (one batch image)
N_CORES = 8
GROUP = 1024
S_X = 127.0 / 4.0
K_OUT = 5.0

_compiled = {}


def build(
    group=GROUP,
    swdge_every=4,         # 1/4 of input via SWDGE cast-DMA: lighter SDMA load
                           # tightens the max-core under co-tenant contention
    xb_bufs=8,
    x8_bufs=5,
    osb_bufs=6,
    ps_bufs=2,             # 2 bufs x 2 names x 2 banks = all 8 PSUM banks
    out_eng="gpsimd",
    taper=True,
    front_taper=(512, 512, 1024),  # small first blocks to prime the pipeline
    act_copy=(0, 2, 4, 6, 8, 10),  # subgroup%12 slots copied by ACT (rest DVE)
    swdge_offset=0,        # block 0 engine-cast (HWDGE in + ACT cast boots
                           # ~1us faster than the SWDGE path)
    pe_warm=10,            # dummy MMs during boot: release the HAM clock gate
):
    import concourse.bass as bass
    import concourse.mybir as mybir
    import concourse.tile as tile
    from concourse import bacc

    f32 = mybir.dt.float32
    bf = mybir.dt.bfloat16
    i8 = mybir.dt.int8
    Copy = mybir.ActivationFunctionType.Copy

    nc = bacc.Bacc(
        "TRN2",
        target_bir_lowering=False,
        debug=False,
        num_devices=N_CORES,
    )
    x_d = nc.declare_dram_parameter("xt8", [CIN, M_CORE], i8, isOutput=False)
    w_d = nc.declare_dram_parameter("Wq", [CIN, COUT], bf, isOutput=False)
    o_d = nc.declare_dram_parameter("out", [COUT, M_CORE], i8, isOutput=True)

    # row blocks: small front blocks prime the pipeline, group-sized body,
    # tapered drain at the tail
    blocks = []
    r = 0
    for fb in front_taper or ():
        blocks.append((r, r + fb))
        r += fb
    while r < M_CORE:
        b_ = min(group, M_CORE - r)
        blocks.append((r, r + b_))
        r += b_
    if taper:
        r0t, r1t = blocks.pop()
        while r1t - r0t > 512:
            mid = r0t + (r1t - r0t) // 2
            mid -= mid % 512
            blocks.append((r0t, mid))
            r0t = mid
        blocks.append((r0t, r1t))

    with tile.TileContext(nc) as tc:
        with (
            tc.tile_pool(name="const", bufs=1) as cpool,
            tc.tile_pool(name="x8", bufs=x8_bufs) as x8pool,
            tc.tile_pool(name="xb", bufs=xb_bufs) as xbpool,
            tc.tile_pool(name="osb", bufs=osb_bufs) as opool,
            tc.tile_pool(name="ps", bufs=ps_bufs, space=bass.MemorySpace.PSUM) as pst,
        ):
            # w_sb[p, a, o] = Wq[a*128 + p, o]; rides the scalar HWDGE queue so
            # the sync queue's first x DMA issues immediately at boot.
            w_sb = cpool.tile([P, 2, COUT], bf)
            nc.scalar.dma_start(
                out=w_sb[:], in_=w_d[:].rearrange("(a p) o -> p a o", p=P)
            )
            if pe_warm:
                # ~4us of dummy matmuls on zeroed SBUF while the first input
                # DMA is in flight: the PE HAM clock gate releases (1.2 ->
                # 2.4 GHz) after ~3.4us of sustained activity, so the real
                # matmuls start warm instead of paying the cold penalty.
                warm = cpool.tile([P, 512], bf)
                nc.vector.memset(warm[:], 0)
                wps = pst.tile([P, 2, 512], f32, name="ps0")
                for _ in range(pe_warm):
                    nc.tensor.matmul(
                        wps[:, 0, :],
                        warm[:, 0:P],
                        warm[:],
                        start=True,
                        stop=True,
                        skip_group_check=True,
                    )
            sg = 0  # global subgroup counter (for copy-engine assignment)
            cast_i = 0  # engine-cast group counter (ACT/DVE alternation)
            for g, (r0, r1) in enumerate(blocks):
                blen = r1 - r0
                src = x_d[:, r0:r1].rearrange("(a p) r -> p a r", p=P)
                xb = xbpool.tile([P, 2, blen], bf)
                # default offset: block 0 rides the SWDGE cast-DMA (no engine
                # cast on the boot critical path). The last blocks always
                # engine-cast so gpsimd's queue retires early and its ~5us
                # dge_drain overlaps the final matmuls.
                off = swdge_every - 1 if swdge_offset is None else swdge_offset
                sel = (g + off) % swdge_every if swdge_every else g % 2
                if g >= len(blocks) - 3:
                    sel = 0
                if swdge_every and sel == swdge_every - 1:
                    # SWDGE cast-DMA: int8 HBM -> bf16 SBUF in one shot
                    nc.gpsimd.dma_start(out=xb[:], in_=src)
                else:
                    x8 = x8pool.tile([P, 2, blen], i8)
                    # (all input on the sync ring: alternating early blocks
                    # onto the scalar ring was measured ~0.5us worse — the
                    # dispatch time on the scalar queue delays ACT's casts)
                    nc.sync.dma_start(out=x8[:], in_=src)
                    # whole-group cast on one engine, alternating per group
                    # (a single writer per tile keeps MM sem-waits minimal —
                    # chunked casts measurably inflate PE active time)
                    if cast_i % 2 == 0:
                        nc.scalar.activation(out=xb[:], in_=x8[:], func=Copy)
                    else:
                        nc.vector.tensor_copy(out=xb[:], in_=x8[:])
                    cast_i += 1
                o_sb = opool.tile([P, 2, blen], i8)
                # process subgroups in pairs sharing each stationary weight
                # load (halves LDWEIGHTS count; accumulation groups interleave
                # across PSUM banks, which the hardware tracks per element).
                # Copies stay per-subgroup on 2-bank tiles: merging them into
                # one 4-bank copy halves PSUM recycling granularity and makes
                # the copy latency pace the PE (measured +20us).
                for s0 in range(0, blen, 1024):
                    nsub = min(2, (blen - s0) // 512)
                    rows = [slice(s0 + k * 512, s0 + (k + 1) * 512) for k in range(nsub)]
                    pss = [
                        pst.tile([P, 2, 512], f32, name=f"ps{k}")
                        for k in range(nsub)
                    ]
                    for oc in range(2):
                        for cc in range(2):
                            for k in range(nsub):
                                nc.tensor.matmul(
                                    pss[k][:, oc, :],
                                    w_sb[:, cc, oc * P : (oc + 1) * P],
                                    xb[:, cc, rows[k]],
                                    start=(cc == 0),
                                    stop=(cc == 1),
                                    skip_group_check=True,
                                )
                    # PSUM f32 -> SBUF int8 (rint+saturate), both oc in one op
                    for k in range(nsub):
                        if (sg % 12) in act_copy:
                            nc.scalar.activation(
                                out=o_sb[:, :, rows[k]], in_=pss[k][:], func=Copy
                            )
                        else:
                            nc.vector.tensor_copy(
                                out=o_sb[:, :, rows[k]], in_=pss[k][:]
                            )
                        sg += 1
                dst = o_d[:, r0:r1].rearrange("(a p) r -> p a r", p=P)
                engs = {
                    "gpsimd": nc.gpsimd,
                    "scalar": nc.scalar,
                    "sync": nc.sync,
                }
                if g >= len(blocks) - 3:
                    # drain edge: HWDGE completion is ~1.4us faster, and
                    # retiring gpsimd's queue early lets its dge_drain
                    # overlap; spread the last DMAs across both HWDGE rings
                    eng = nc.scalar if (len(blocks) - 1 - g) % 2 == 0 else nc.sync
                elif isinstance(out_eng, (list, tuple)):
                    eng = engs[out_eng[g % len(out_eng)]]
                else:
                    eng = engs[out_eng]
                eng.dma_start(out=dst, in_=o_sb[:])
    nc.compile()
    return nc


def _get_compiled(key="full", **kwargs):
    if key not in _compiled:
        _compiled[key] = build(**kwargs)
    return _compiled[key]


def _prep_inputs(x_shards, W):
    """x_shards: [n, M_CORE, CIN] f32 -> int8 channels-major per core + folded W."""
    n = x_shards.shape[0]
    xq = np.clip(np.rint(x_shards * S_X), -127, 127).astype(np.int8)
    xt8 = np.empty((n, CIN, M_CORE), dtype=np.int8)
    for i in range(n):
        np.copyto(xt8[i], xq[i].T)
    W = np.ascontiguousarray(W, dtype=np.float32)
    sigma = np.linalg.norm(W, axis=0)
    t = (127.0 / (K_OUT * sigma)).astype(np.float32)  # [COUT]
    Wq = (W * (t[None, :] / S_X)).astype(BF16)
    return xt8, Wq, t


def run_spmd(nc, x_shards, W, trace=False, **kwargs):
    """x_shards: [n_cores, M_CORE, CIN] f32. Returns (stacked f32 outs, results)."""
    from concourse.bass_utils import run_bass_kernel_spmd

    n = x_shards.shape[0]
    xt8, Wq, t = _prep_inputs(x_shards, W)
    in_maps = [{"xt8": xt8[i], "Wq": Wq} for i in range(n)]
    res = run_bass_kernel_spmd(
        nc, in_maps, core_ids=list(range(n)), trace=trace, **kwargs
    )
    inv_t = (1.0 / t).astype(np.float32)  # [COUT]
    outs = np.empty((n, M_CORE, COUT), dtype=np.float32)
    for i in range(n):
        o8 = np.asarray(res.results[i]["out"])  # [COUT, M_CORE] int8
        np.multiply(o8.T.astype(np.float32), inv_t[None, :], out=outs[i])
    return outs, res


def kernel(x, W):
    x = np.ascontiguousarray(x, dtype=np.float32).reshape(N_CORES, M_CORE, CIN)
    W = np.ascontiguousarray(W, dtype=np.float32)
    nc = _get_compiled("full")
    outs, _ = run_spmd(nc, x, W)
    return outs.reshape(B, H, Wdim, COUT)


# revision 46
# speedup vs baseline: 1.0047x; 1.0047x over previous
"""Trainium2 Bass kernel for ChannelProjector2D: out[b,h,w,o] = x[b,h,w,c] @ W[c,o].

Strategy (data-parallel, one batch image per NeuronCore; int8 I/O):
  - Error gate is rel_err < 2e-2. bf16 I/O gives 2.9e-3 but is DMA-bound at
    ~150-173us (51.4 MB/core through the SDMA engines at ~330 GB/s). The
    binding resource is SDMA-processed bytes, so both directions go int8:
      x: host-quantized  xq = clip(rint(x*31.75), -127, 127)  (s_x = 127/4)
      W: host-folded     Wq[c,o] = bf16(W[c,o] * t_o / s_x),  t_o = 127/(5*sigma_o)
      out: PSUM (= t_o * out_o) stored rint->int8 saturating; host divides by t_o.
    Probed on HW: ACT/DVE f32->int8 stores round-to-nearest-even + saturate,
    and SWDGE cast-DMA int8->bf16 is an exact value cast. rel_err 1.486e-2
    (measured HW == numpy sim on the exact generator data; 46 sat elts).
  - Per core: int8 x rows channels-major (c on partitions, 2 chunks of 128).
    Alternating 1024-row groups: (a) HWDGE int8 DMA + one whole-group
    int8->bf16 cast on ACT or DVE (single writer per tile -- chunked/multi-
    writer casts inflate PE sem-wait time ~45ns/MM, measured); (b) gpsimd
    SWDGE cast-DMA straight to bf16 (costs 2B/elt of SDMA-processed bytes
    but zero engine time). bf16 matmuls (2cc x 2oc per 512 rows, pairs of
    subgroups interleaved across 2-bank PSUM tiles) accumulate f32; a single
    [128,2,512] PSUM->SBUF int8 copy per subgroup alternates ACT/DVE
    (act_copy slots balance the 1.2 vs 0.96 GHz engines). Out-DMA int8 on
    the SWDGE ring; the last 3 blocks drain on the HWDGE rings (lower
    completion latency, lets gpsimd's 5us dge_drain overlap). ~10 dummy
    matmuls on zeroed SBUF during boot release the PE HAM clock gate
    (1.2->2.4 GHz) before real work arrives; small front blocks prime the
    pipeline.
  - Measured (NTFF, 8 cores): ~109-110us mean, ~111-112.5us max-core
    (swdge_every=4 trades ~0.3us mean for a tighter max under co-tenant HBM
    contention), vs 146-157us for the bf16 baseline. Breakdown: PE busy
    ~90us (384 MM x ~225ns, N=512 warm + LDW residual; 4 cyc/row = 83.6us
    floor), ACT ~80, DVE ~75; boot ~12us (Tile preamble ~7 + first DMA) and
    tail ~9.5us (copy + DMA receipt ~2 + 5-engine exit-barrier ring ~4.5)
    bracket the span. fp8 x fails the gate (2.7e-2); int8 is the traffic
    floor since matmul needs >=8-bit x, and PE at 4 cyc/row is then the
    wall. Merging the per-subgroup PSUM copies into one 4-bank copy halves
    PSUM recycling granularity and regresses ~20us — keep 2-bank tiles.
"""

import numpy as np
import ml_dtypes

BF16 = ml_dtypes.bfloat16

P = 128
CIN = 256
COUT = 256
B, H, Wdim = 8, 224, 224
M_CORE = H * Wdim          # 50176 rows per core (one batch image)
N_CORES = 8
GROUP = 1024
S_X = 127.0 / 4.0
K_OUT = 5.0

_compiled = {}


def build(
    group=GROUP,
    swdge_every=4,         # 1/4 of input via SWDGE cast-DMA: lighter SDMA load
                           # tightens the max-core under co-tenant contention
    xb_bufs=8,
    x8_bufs=5,
    osb_bufs=6,
    ps_bufs=2,             # 2 bufs x 2 names x 2 banks = all 8 PSUM banks
    out_eng="gpsimd",
    taper=True,
    front_taper=(512, 512, 1024),  # small first blocks to prime the pipeline
    act_copy=(0, 2, 4, 6, 8, 10),  # subgroup%12 slots copied by ACT (rest DVE)
    swdge_offset=0,        # block 0 engine-cast (HWDGE in + ACT cast boots
                           # ~1us faster than the SWDGE path)
    pe_warm=10,            # dummy MMs during boot: release the HAM clock gate
):
    import concourse.bass as bass
    import concourse.mybir as mybir
    import concourse.tile as tile
    from concourse import bacc

    f32 = mybir.dt.float32
    bf = mybir.dt.bfloat16
    i8 = mybir.dt.int8
    Copy = mybir.ActivationFunctionType.Copy

    nc = bacc.Bacc(
        "TRN2",
        target_bir_lowering=False,
        debug=False,
        num_devices=N_CORES,
    )
    x_d = nc.declare_dram_parameter("xt8", [CIN, M_CORE], i8, isOutput=False)
    w_d = nc.declare_dram_parameter("Wq", [CIN, COUT], bf, isOutput=False)
    o_d = nc.declare_dram_parameter("out", [COUT, M_CORE], i8, isOutput=True)

    # row blocks: small front blocks prime the pipeline, group-sized body,
    # tapered drain at the tail
    blocks = []
    r = 0
    for fb in front_taper or ():
        blocks.append((r, r + fb))
        r += fb
    while r < M_CORE:
        b_ = min(group, M_CORE - r)
        blocks.append((r, r + b_))
        r += b_
    if taper:
        r0t, r1t = blocks.pop()
        while r1t - r0t > 512:
            mid = r0t + (r1t - r0t) // 2
            mid -= mid % 512
            blocks.append((r0t, mid))
            r0t = mid
        blocks.append((r0t, r1t))

    with tile.TileContext(nc) as tc:
        with (
            tc.tile_pool(name="const", bufs=1) as cpool,
            tc.tile_pool(name="x8", bufs=x8_bufs) as x8pool,
            tc.tile_pool(name="xb", bufs=xb_bufs) as xbpool,
            tc.tile_pool(name="osb", bufs=osb_bufs) as opool,
            tc.tile_pool(name="ps", bufs=ps_bufs, space=bass.MemorySpace.PSUM) as pst,
        ):
            # w_sb[p, a, o] = Wq[a*128 + p, o]; rides the scalar HWDGE queue so
            # the sync queue's first x DMA issues immediately at boot.
            w_sb = cpool.tile([P, 2, COUT], bf)
            nc.scalar.dma_start(
                out=w_sb[:], in_=w_d[:].rearrange("(a p) o -> p a o", p=P)
            )
            if pe_warm:
                # ~4us of dummy matmuls on zeroed SBUF while the first input
                # DMA is in flight: the PE HAM clock gate releases (1.2 ->
                # 2.4 GHz) after ~3.4us of sustained activity, so the real
                # matmuls start warm instead of paying the cold penalty.
                warm = cpool.tile([P, 512], bf)
                nc.vector.memset(warm[:], 0)
                wps = pst.tile([P, 2, 512], f32, name="ps0")
                for _ in range(pe_warm):
                    nc.tensor.matmul(
                        wps[:, 0, :],
                        warm[:, 0:P],
                        warm[:],
                        start=True,
                        stop=True,
                        skip_group_check=True,
                    )
            sg = 0  # global subgroup counter (for copy-engine assignment)
            cast_i = 0  # engine-cast group counter (ACT/DVE alternation)
            for g, (r0, r1) in enumerate(blocks):
                blen = r1 - r0
                src = x_d[:, r0:r1].rearrange("(a p) r -> p a r", p=P)
                xb = xbpool.tile([P, 2, blen], bf)
                # default offset: block 0 rides the SWDGE cast-DMA (no engine
                # cast on the boot critical path). The last blocks always
                # engine-cast so gpsimd's queue retires early and its ~5us
                # dge_drain overlaps the final matmuls.
                off = swdge_every - 1 if swdge_offset is None else swdge_offset
                sel = (g + off) % swdge_every if swdge_every else g % 2
                if g >= len(blocks) - 3:
                    sel = 0
                if swdge_every and sel == swdge_every - 1:
                    # SWDGE cast-DMA: int8 HBM -> bf16 SBUF in one shot
                    nc.gpsimd.dma_start(out=xb[:], in_=src)
                else:
                    x8 = x8pool.tile([P, 2, blen], i8)
                    # (all input on the sync ring: alternating early blocks
                    # onto the scalar ring was measured ~0.5us worse — the
                    # dispatch time on the scalar queue delays ACT's casts)
                    nc.sync.dma_start(out=x8[:], in_=src)
                    # whole-group cast on one engine, alternating per group
                    # (a single writer per tile keeps MM sem-waits minimal —
                    # chunked casts measurably inflate PE active time)
                    if cast_i % 2 == 0:
                        nc.scalar.activation(out=xb[:], in_=x8[:], func=Copy)
                    else:
                        nc.vector.tensor_copy(out=xb[:], in_=x8[:])
                    cast_i += 1
                o_sb = opool.tile([P, 2, blen], i8)
                # process subgroups in pairs sharing each stationary weight
                # load (halves LDWEIGHTS count; accumulation groups interleave
                # across PSUM banks, which the hardware tracks per element).
                # Copies stay per-subgroup on 2-bank tiles: merging them into
                # one 4-bank copy halves PSUM recycling granularity and makes
                # the copy latency pace the PE (measured +20us).
                for s0 in range(0, blen, 1024):
                    nsub = min(2, (blen - s0) // 512)
                    rows = [slice(s0 + k * 512, s0 + (k + 1) * 512) for k in range(nsub)]
                    pss = [
                        pst.tile([P, 2, 512], f32, name=f"ps{k}")
                        for k in range(nsub)
                    ]
                    for oc in range(2):
                        for cc in range(2):
                            for k in range(nsub):
                                nc.tensor.matmul(
                                    pss[k][:, oc, :],
                                    w_sb[:, cc, oc * P : (oc + 1) * P],
                                    xb[:, cc, rows[k]],
                                    start=(cc == 0),
                                    stop=(cc == 1),
                                    skip_group_check=True,
                                )
                    # PSUM f32 -> SBUF int8 (rint+saturate), both oc in one op
                    for k in range(nsub):
                        if (sg % 12) in act_copy:
                            nc.scalar.activation(
                                out=o_sb[:, :, rows[k]], in_=pss[k][:], func=Copy
                            )
                        else:
                            nc.vector.tensor_copy(
                                out=o_sb[:, :, rows[k]], in_=pss[k][:]
                            )
                        sg += 1
                dst = o_d[:, r0:r1].rearrange("(a p) r -> p a r", p=P)
                engs = {
                    "gpsimd": nc.gpsimd,
                    "scalar": nc.scalar,
                    "sync": nc.sync,
                }
                if g >= len(blocks) - 3:
                    # drain edge: HWDGE completion is ~1.4us faster, and
                    # retiring gpsimd's queue early lets its dge_drain
                    # overlap; spread the last DMAs across both HWDGE rings
                    eng = nc.scalar if (len(blocks) - 1 - g) % 2 == 0 else nc.sync
                elif isinstance(out_eng, (list, tuple)):
                    eng = engs[out_eng[g % len(out_eng)]]
                else:
                    eng = engs[out_eng]
                eng.dma_start(out=dst, in_=o_sb[:])
    nc.compile()
    return nc


def _get_compiled(key="full", **kwargs):
    if key not in _compiled:
        _compiled[key] = build(**kwargs)
    return _compiled[key]


def _prep_inputs(x_shards, W):
    """x_shards: [n, M_CORE, CIN] f32 -> int8 channels-major per core + folded W."""
    n = x_shards.shape[0]
    xq = np.clip(np.rint(x_shards * S_X), -127, 127).astype(np.int8)
    xt8 = np.empty((n, CIN, M_CORE), dtype=np.int8)
    for i in range(n):
        np.copyto(xt8[i], xq[i].T)
    W = np.ascontiguousarray(W, dtype=np.float32)
    sigma = np.linalg.norm(W, axis=0)
    t = (127.0 / (K_OUT * sigma)).astype(np.float32)  # [COUT]
    Wq = (W * (t[None, :] / S_X)).astype(BF16)
    return xt8, Wq, t


def run_spmd(nc, x_shards, W, trace=False, **kwargs):
    """x_shards: [n_cores, M_CORE, CIN] f32. Returns (stacked f32 outs, results)."""
    from concourse.bass_utils import run_bass_kernel_spmd

    n = x_shards.shape[0]
    xt8, Wq, t = _prep_inputs(x_shards, W)
    in_maps = [{"xt8": xt8[i], "Wq": Wq} for i in range(n)]
    res = run_bass_kernel_spmd(
        nc, in_maps, core_ids=list(range(n)), trace=trace, **kwargs
    )
    inv_t = (1.0 / t).astype(np.float32)  # [COUT]
    outs = np.empty((n, M_CORE, COUT), dtype=np.float32)
    for i in range(n):
        o8 = np.asarray(res.results[i]["out"])  # [COUT, M_CORE] int8
        np.multiply(o8.T.astype(np.float32), inv_t[None, :], out=outs[i])
    return outs, res


def kernel(x, W):
    x = np.ascontiguousarray(x, dtype=np.float32).reshape(N_CORES, M_CORE, CIN)
    W = np.ascontiguousarray(W, dtype=np.float32)
    nc = _get_compiled("full")
    outs, _ = run_spmd(nc, x, W)
    return outs.reshape(B, H, Wdim, COUT)


# revision 48
# speedup vs baseline: 1.0076x; 1.0030x over previous
"""Trainium2 Bass kernel for ChannelProjector2D: out[b,h,w,o] = x[b,h,w,c] @ W[c,o].

Strategy (data-parallel, one batch image per NeuronCore; int8 I/O):
  - Error gate is rel_err < 2e-2. bf16 I/O gives 2.9e-3 but is DMA-bound at
    ~150-173us (51.4 MB/core through the SDMA engines at ~330 GB/s). The
    binding resource is SDMA-processed bytes, so both directions go int8:
      x: host-quantized  xq = clip(rint(x*31.75), -127, 127)  (s_x = 127/4)
      W: host-folded     Wq[c,o] = bf16(W[c,o] * t_o / s_x),  t_o = 127/(5*sigma_o)
      out: PSUM (= t_o * out_o) stored rint->int8 saturating; host divides by t_o.
    Probed on HW: ACT/DVE f32->int8 stores round-to-nearest-even + saturate,
    and SWDGE cast-DMA int8->bf16 is an exact value cast. rel_err 1.486e-2
    (measured HW == numpy sim on the exact generator data; 46 sat elts).
  - Per core: int8 x rows channels-major (c on partitions, 2 chunks of 128).
    Alternating 1024-row groups: (a) HWDGE int8 DMA + one whole-group
    int8->bf16 cast on ACT or DVE (single writer per tile -- chunked/multi-
    writer casts inflate PE sem-wait time ~45ns/MM, measured); (b) gpsimd
    SWDGE cast-DMA straight to bf16 (costs 2B/elt of SDMA-processed bytes
    but zero engine time). bf16 matmuls (2cc x 2oc per 512 rows, pairs of
    subgroups interleaved across 2-bank PSUM tiles) accumulate f32; a single
    [128,2,512] PSUM->SBUF int8 copy per subgroup alternates ACT/DVE
    (act_copy slots balance the 1.2 vs 0.96 GHz engines). Out-DMA int8 on
    the SWDGE ring; the last 3 blocks drain on the HWDGE rings (lower
    completion latency, lets gpsimd's 5us dge_drain overlap). ~10 dummy
    matmuls on zeroed SBUF during boot release the PE HAM clock gate
    (1.2->2.4 GHz) before real work arrives; small front blocks prime the
    pipeline.
  - Measured (NTFF, 8 cores): ~109-110us mean, ~111-112.5us max-core
    (swdge_every=4 trades ~0.3us mean for a tighter max under co-tenant HBM
    contention), vs 146-157us for the bf16 baseline. Breakdown: PE busy
    ~90us (384 MM x ~225ns, N=512 warm + LDW residual; 4 cyc/row = 83.6us
    floor), ACT ~80, DVE ~75; boot ~12us (Tile preamble ~7 + first DMA) and
    tail ~9.5us (copy + DMA receipt ~2 + 5-engine exit-barrier ring ~4.5)
    bracket the span. fp8 x fails the gate (2.7e-2); int8 is the traffic
    floor since matmul needs >=8-bit x, and PE at 4 cyc/row is then the
    wall. Merging the per-subgroup PSUM copies into one 4-bank copy halves
    PSUM recycling granularity and regresses ~20us — keep 2-bank tiles.
"""

import numpy as np
import ml_dtypes

BF16 = ml_dtypes.bfloat16

P = 128
CIN = 256
COUT = 256
B, H, Wdim = 8, 224, 224
M_CORE = H * Wdim          # 50176 rows per core (one batch image)
N_CORES = 8
GROUP = 1024
S_X = 127.0 / 4.0
K_OUT = 5.0

_compiled = {}


def build(
    group=GROUP,
    swdge_every=4,         # 1/4 of input via SWDGE cast-DMA: lighter SDMA load
                           # tightens the max-core under co-tenant contention
    xb_bufs=8,
    x8_bufs=5,
    osb_bufs=6,
    ps_bufs=2,             # 2 bufs x 2 names x 2 banks = all 8 PSUM banks
    out_eng="gpsimd",
    taper=True,
    front_taper=(512, 512, 1024),  # small first blocks to prime the pipeline
    act_copy=(0, 2, 4, 6, 8, 10),  # subgroup%12 slots copied by ACT (rest DVE)
    swdge_offset=0,        # block 0 engine-cast (HWDGE in + ACT cast boots
                           # ~1us faster than the SWDGE path)
    pe_warm=10,            # dummy MMs during boot: release the HAM clock gate
):
    import concourse.bass as bass
    import concourse.mybir as mybir
    import concourse.tile as tile
    from concourse import bacc

    f32 = mybir.dt.float32
    bf = mybir.dt.bfloat16
    i8 = mybir.dt.int8
    Copy = mybir.ActivationFunctionType.Copy

    nc = bacc.Bacc(
        "TRN2",
        target_bir_lowering=False,
        debug=False,
        num_devices=N_CORES,
    )
    x_d = nc.declare_dram_parameter("xt8", [CIN, M_CORE], i8, isOutput=False)
    w_d = nc.declare_dram_parameter("Wq", [CIN, COUT], bf, isOutput=False)
    o_d = nc.declare_dram_parameter("out", [COUT, M_CORE], i8, isOutput=True)

    # row blocks: small front blocks prime the pipeline, group-sized body,
    # tapered drain at the tail
    blocks = []
    r = 0
    for fb in front_taper or ():
        blocks.append((r, r + fb))
        r += fb
    while r < M_CORE:
        b_ = min(group, M_CORE - r)
        blocks.append((r, r + b_))
        r += b_
    if taper:
        r0t, r1t = blocks.pop()
        while r1t - r0t > 512:
            mid = r0t + (r1t - r0t) // 2
            mid -= mid % 512
            blocks.append((r0t, mid))
            r0t = mid
        blocks.append((r0t, r1t))

    with tile.TileContext(nc) as tc:
        with (
            tc.tile_pool(name="const", bufs=1) as cpool,
            tc.tile_pool(name="x8", bufs=x8_bufs) as x8pool,
            tc.tile_pool(name="xb", bufs=xb_bufs) as xbpool,
            tc.tile_pool(name="osb", bufs=osb_bufs) as opool,
            tc.tile_pool(name="ps", bufs=ps_bufs, space=bass.MemorySpace.PSUM) as pst,
        ):
            # w_sb[p, a, o] = Wq[a*128 + p, o]; rides the scalar HWDGE queue so
            # the sync queue's first x DMA issues immediately at boot.
            w_sb = cpool.tile([P, 2, COUT], bf)
            nc.scalar.dma_start(
                out=w_sb[:], in_=w_d[:].rearrange("(a p) o -> p a o", p=P)
            )
            if pe_warm:
                # ~4us of dummy matmuls on zeroed SBUF while the first input
                # DMA is in flight: the PE HAM clock gate releases (1.2 ->
                # 2.4 GHz) after ~3.4us of sustained activity, so the real
                # matmuls start warm instead of paying the cold penalty.
                warm = cpool.tile([P, 512], bf)
                nc.vector.memset(warm[:], 0)
                wps = pst.tile([P, 2, 512], f32, name="ps0")
                for _ in range(pe_warm):
                    nc.tensor.matmul(
                        wps[:, 0, :],
                        warm[:, 0:P],
                        warm[:],
                        start=True,
                        stop=True,
                        skip_group_check=True,
                    )
            sg = 0  # global subgroup counter (for copy-engine assignment)
            cast_i = 0  # engine-cast group counter (ACT/DVE alternation)
            for g, (r0, r1) in enumerate(blocks):
                blen = r1 - r0
                src = x_d[:, r0:r1].rearrange("(a p) r -> p a r", p=P)
                xb = xbpool.tile([P, 2, blen], bf)
                # default offset: block 0 rides the SWDGE cast-DMA (no engine
                # cast on the boot critical path). The last blocks always
                # engine-cast so gpsimd's queue retires early and its ~5us
                # dge_drain overlaps the final matmuls.
                off = swdge_every - 1 if swdge_offset is None else swdge_offset
                sel = (g + off) % swdge_every if swdge_every else g % 2
                if g >= len(blocks) - 3:
                    sel = 0
                if swdge_every and sel == swdge_every - 1:
                    # SWDGE cast-DMA: int8 HBM -> bf16 SBUF in one shot
                    nc.gpsimd.dma_start(out=xb[:], in_=src)
                else:
                    x8 = x8pool.tile([P, 2, blen], i8)
                    # (all input on the sync ring: alternating early blocks
                    # onto the scalar ring was measured ~0.5us worse — the
                    # dispatch time on the scalar queue delays ACT's casts)
                    nc.sync.dma_start(out=x8[:], in_=src)
                    # whole-group cast on one engine, alternating per group
                    # (a single writer per tile keeps MM sem-waits minimal —
                    # chunked casts measurably inflate PE active time)
                    if cast_i % 2 == 0:
                        nc.scalar.activation(out=xb[:], in_=x8[:], func=Copy)
                    else:
                        nc.vector.tensor_copy(out=xb[:], in_=x8[:])
                    cast_i += 1
                o_sb = opool.tile([P, 2, blen], i8)
                # process subgroups in pairs sharing each stationary weight
                # load (halves LDWEIGHTS count; accumulation groups interleave
                # across PSUM banks, which the hardware tracks per element).
                # Copies stay per-subgroup on 2-bank tiles: merging them into
                # one 4-bank copy halves PSUM recycling granularity and makes
                # the copy latency pace the PE (measured +20us).
                for s0 in range(0, blen, 1024):
                    nsub = min(2, (blen - s0) // 512)
                    rows = [slice(s0 + k * 512, s0 + (k + 1) * 512) for k in range(nsub)]
                    pss = [
                        pst.tile([P, 2, 512], f32, name=f"ps{k}")
                        for k in range(nsub)
                    ]
                    for oc in range(2):
                        for cc in range(2):
                            for k in range(nsub):
                                nc.tensor.matmul(
                                    pss[k][:, oc, :],
                                    w_sb[:, cc, oc * P : (oc + 1) * P],
                                    xb[:, cc, rows[k]],
                                    start=(cc == 0),
                                    stop=(cc == 1),
                                    skip_group_check=True,
                                )
                    # PSUM f32 -> SBUF int8 (rint+saturate), both oc in one op
                    for k in range(nsub):
                        if g == len(blocks) - 1:
                            # drain edge: split the final copy across both
                            # engines in parallel to shorten the tail chain
                            nc.scalar.activation(
                                out=o_sb[:, :, s0 + k * 512 : s0 + k * 512 + 256],
                                in_=pss[k][:, :, 0:256],
                                func=Copy,
                            )
                            nc.vector.tensor_copy(
                                out=o_sb[:, :, s0 + k * 512 + 256 : s0 + (k + 1) * 512],
                                in_=pss[k][:, :, 256:512],
                            )
                        elif (sg % 12) in act_copy:
                            nc.scalar.activation(
                                out=o_sb[:, :, rows[k]], in_=pss[k][:], func=Copy
                            )
                        else:
                            nc.vector.tensor_copy(
                                out=o_sb[:, :, rows[k]], in_=pss[k][:]
                            )
                        sg += 1
                dst = o_d[:, r0:r1].rearrange("(a p) r -> p a r", p=P)
                engs = {
                    "gpsimd": nc.gpsimd,
                    "scalar": nc.scalar,
                    "sync": nc.sync,
                }
                if g == len(blocks) - 1:
                    # final block: both HWDGE rings move half each so the
                    # last completion sem fires as early as possible
                    nc.scalar.dma_start(
                        out=dst[:, :, 0 : blen // 2], in_=o_sb[:, :, 0 : blen // 2]
                    )
                    nc.sync.dma_start(
                        out=dst[:, :, blen // 2 : blen],
                        in_=o_sb[:, :, blen // 2 : blen],
                    )
                elif g >= len(blocks) - 3:
                    # drain edge: HWDGE completion is ~1.4us faster, and
                    # retiring gpsimd's queue early lets its dge_drain
                    # overlap; spread the last DMAs across both HWDGE rings
                    eng = nc.scalar if (len(blocks) - 1 - g) % 2 == 0 else nc.sync
                    eng.dma_start(out=dst, in_=o_sb[:])
                elif isinstance(out_eng, (list, tuple)):
                    engs[out_eng[g % len(out_eng)]].dma_start(out=dst, in_=o_sb[:])
                else:
                    engs[out_eng].dma_start(out=dst, in_=o_sb[:])
    nc.compile()
    return nc


def _get_compiled(key="full", **kwargs):
    if key not in _compiled:
        _compiled[key] = build(**kwargs)
    return _compiled[key]


def _prep_inputs(x_shards, W):
    """x_shards: [n, M_CORE, CIN] f32 -> int8 channels-major per core + folded W."""
    n = x_shards.shape[0]
    xq = np.clip(np.rint(x_shards * S_X), -127, 127).astype(np.int8)
    xt8 = np.empty((n, CIN, M_CORE), dtype=np.int8)
    for i in range(n):
        np.copyto(xt8[i], xq[i].T)
    W = np.ascontiguousarray(W, dtype=np.float32)
    sigma = np.linalg.norm(W, axis=0)
    t = (127.0 / (K_OUT * sigma)).astype(np.float32)  # [COUT]
    Wq = (W * (t[None, :] / S_X)).astype(BF16)
    return xt8, Wq, t


def run_spmd(nc, x_shards, W, trace=False, **kwargs):
    """x_shards: [n_cores, M_CORE, CIN] f32. Returns (stacked f32 outs, results)."""
    from concourse.bass_utils import run_bass_kernel_spmd

    n = x_shards.shape[0]
    xt8, Wq, t = _prep_inputs(x_shards, W)
    in_maps = [{"xt8": xt8[i], "Wq": Wq} for i in range(n)]
    res = run_bass_kernel_spmd(
        nc, in_maps, core_ids=list(range(n)), trace=trace, **kwargs
    )
    inv_t = (1.0 / t).astype(np.float32)  # [COUT]
    outs = np.empty((n, M_CORE, COUT), dtype=np.float32)
    for i in range(n):
        o8 = np.asarray(res.results[i]["out"])  # [COUT, M_CORE] int8
        np.multiply(o8.T.astype(np.float32), inv_t[None, :], out=outs[i])
    return outs, res


def kernel(x, W):
    x = np.ascontiguousarray(x, dtype=np.float32).reshape(N_CORES, M_CORE, CIN)
    W = np.ascontiguousarray(W, dtype=np.float32)
    nc = _get_compiled("full")
    outs, _ = run_spmd(nc, x, W)
    return outs.reshape(B, H, Wdim, COUT)
